# revision 14
# baseline (speedup 1.0000x reference)
"""Trainium2 Bass kernel for nn_CrossBlock (pre-LN self-attn + cross-attn + MLP).

Sharding: 8 cores = 2 (batch) x 4 (query-token slices of 512). No collectives:
each core computes K/V over the full 2048 keys of its batch (replicated across
the 4 cores sharing a batch) and produces its own 512-token slice of the output.

Device layout: activations are kept feature-major ("transposed", [C, tokens])
so every projection chains through the TensorEngine without transposes:
    Y^T = W.T @ X^T   (lhsT = W[cin, cout] chunks, rhs = X^T chunks)
Attention scores are computed as S^T [keys, queries]; softmax denominators come
from an extra ones-column appended to V (token-major), so the AV matmul yields
[dh + 1, q] with the last row = sum_k exp(s). Normalization is a per-query
(free-dim) multiply with a partition-broadcast reciprocal. No max-subtraction:
scores/sqrt(dh) are O(1) for this problem's scale.

LayerNorm gain/bias are folded into the following projection weights on the
host. LN on device is then just (x - mu) * rstd with stats computed
feature-major via ones-matmul column sums (results naturally broadcast over all
128 partitions); rstd = exp(-0.5*ln(var+eps)) keeps the ScalarEngine in the
exp/ln table set shared with softmax.

Matmuls run as float32r (FP22 multiplies, fp32 accumulate) - full PE rate.
"""

import contextlib
import math

import numpy as np

import concourse.bass as bass
import concourse.tile as tile
from concourse import bacc, mybir
from concourse.bass_utils import run_bass_kernel_spmd

# Problem constants (hardcoded per contract)
C = 768
H = 12
B = 2
TX = 2048
TC = 2048
DH = 64
P = 128
KC = C // P          # 6 cin/cout chunks of 128
TOWN = TX // 4       # 512 query tokens per core
NT_FULL = TC // 512  # 4 key-token slices of 512
TKC = TC // P        # 16 key-token chunks of 128
H1 = 4 * C           # 3072
MC1 = H1 // P        # 24 chunks of mlp hidden

F32 = mybir.dt.float32
F32R = mybir.dt.float32r
AF = mybir.ActivationFunctionType
OP = mybir.AluOpType

# S-psum group sizes per head (sum = TKC); 3-bank S tiles double-buffered plus
# two Y tiles fit the 8 PSUM banks.
SGROUPS = (3, 3, 3, 3, 2, 2)


def _r(ap):
    """View an fp32 AP as float32r for full-rate PE matmuls."""
    return ap.bitcast(mybir.dt.float32r)


def _pbcast(ap, nparts):
    """Partition-broadcast AP: read a [1, ...] AP as [nparts, ...] (step 0)."""
    return bass.AP(tensor=ap.tensor, offset=ap.offset, ap=[[0, nparts]] + ap.ap[1:])


def _fbcast(col, dims):
    """Free-dim broadcast AP: read a [P, 1] AP as [P, *dims] (step 0)."""
    return bass.AP(tensor=col.tensor, offset=col.offset,
                   ap=[col.ap[0]] + [[0, d] for d in dims])


class _Prog:
    """Builds the single SPMD program shared by all 8 cores."""

    def __init__(self, bias_nz):
        self.bias_nz = bias_nz  # dict name -> bool (nonzero bias present)
        self.nc = bacc.Bacc("TRN2", target_bir_lowering=False, debug=False)
        self._build()

    # ---------- helpers ----------

    def _copyback(self, out, psum, bias_col=None):
        """PSUM -> SBUF copyback, optionally + per-partition bias column."""
        nc = self.nc
        if bias_col is not None:
            nc.vector.tensor_scalar(out, psum, bias_col, None, OP.add)
        else:
            nc.vector.tensor_copy(out=out, in_=psum)

    def _ln_slice(self, xT, out, psum_pool, eps=1e-5):
        """Feature-major layernorm of one 512-token slice (gain/bias folded
        into downstream weights). xT/out: [P, KC, 512] SBUF (may alias)."""
        nc = self.nc
        ps_sum = psum_pool.tile([P, 512], F32, tag="ln_sum", bufs=1)
        ps_sq = psum_pool.tile([P, 512], F32, tag="ln_sq", bufs=1)
        for j in range(KC):
            nc.tensor.matmul(ps_sum, _r(self.ones[:]), _r(xT[:, j, :]),
                             start=(j == 0), stop=(j == KC - 1))
        for j in range(KC):
            sq = self.lntmp.tile([P, 512], F32R, tag="ln_scr")
            nc.vector.tensor_tensor(sq[:], xT[:, j, :], xT[:, j, :], OP.mult)
            nc.tensor.matmul(ps_sq, _r(self.ones[:]), _r(sq[:]),
                             start=(j == 0), stop=(j == KC - 1))
        mu = self.lntmp.tile([P, 512], F32, tag="ln_mu")
        nc.vector.tensor_scalar(mu[:], ps_sum, 1.0 / C, None, OP.mult)
        var = self.lntmp.tile([P, 512], F32, tag="ln_var")
        # var + eps = (sumsq/C + eps) - mu^2
        nc.vector.tensor_scalar(var[:], ps_sq, 1.0 / C, eps, OP.mult, OP.add)
        mu2 = self.lntmp.tile([P, 512], F32, tag="ln_scr")
        nc.vector.tensor_tensor(mu2[:], mu[:], mu[:], OP.mult)
        nc.vector.tensor_tensor(var[:], var[:], mu2[:], OP.subtract)
        # var <- rstd = exp(-0.5 * ln(var+eps)); stays in the exp/ln table set
        nc.scalar.activation(var[:], var[:], AF.Ln)
        nc.scalar.activation(var[:], var[:], AF.Exp, scale=-0.5)
        for j in range(KC):
            d = self.lntmp.tile([P, 512], F32, tag="ln_scr")
            nc.vector.tensor_tensor(d[:], xT[:, j, :], mu[:], OP.subtract)
            nc.vector.tensor_tensor(out[:, j, :], d[:], var[:], OP.mult)

    def _load_w(self, dram_ap, tag="w"):
        """Load a [cin, cout]-chunked weight view into SBUF [P, ko, co]."""
        _, ko, co = dram_ap.shape
        w = self.wpool.tile([P, ko, co], F32R, tag=tag)
        self.nc.sync.dma_start(out=w[:], in_=dram_ap)
        return w

    def _wview(self, dram, kchunks):
        return dram.ap().rearrange("(ko p) co -> p ko co", p=P)

    def _bias_cols(self, name, nchunks):
        """Load bias vector as [P, nchunks] (feature-per-partition), or None."""
        if not self.bias_nz[name]:
            return None
        b = self.nc.dram_tensor(name, [nchunks * P], F32, kind="ExternalInput")
        t = self.biaspool.tile([P, nchunks], F32, tag=f"b_{name}")
        self.nc.sync.dma_start(
            out=t[:], in_=b.ap().rearrange("(ko p) -> p ko", p=P))
        return t

    def _bias_bcast(self, name, n):
        """Load bias vector as [P, n] broadcast over partitions, or None."""
        if not self.bias_nz[name]:
            return None
        b = self.nc.dram_tensor(name, [n], F32, kind="ExternalInput")
        t = self.biaspool.tile([P, n], F32, tag=f"bb_{name}")
        self.nc.sync.dma_start(out=t[:], in_=_pbcast(b.ap()[None, :], P))
        return t

    def _attn_stage(self, tc, src_dram, normalize, wq_d, wk_d, wv_d, wo_d,
                    bq, bk, bv_name, bo, xres):
        """One attention stage (self or cross), accumulated into xres.

        src_dram: DRAM [C, TC] feature-major K/V source (x^T for self-attn --
        normalized on the fly; raw context^T for cross-attn).
        """
        nc = self.nc
        src_r = src_dram.ap().rearrange("(ko p) t -> p ko t", p=P)
        bv_b = self._bias_bcast(bv_name, C)

        with contextlib.ExitStack() as st:
            apool = st.enter_context(tc.tile_pool(name="attn_big", bufs=1))
            tpool = st.enter_context(tc.tile_pool(name="attn_t12", bufs=2))

            kfull = apool.tile([P, KC, TC], F32R, tag="K_full")
            vfull = apool.tile([P, TKC, H, DH + 1], F32R, tag="V_full")
            nc.vector.tensor_copy(out=vfull[:, :, :, DH:DH + 1],
                                  in_=_fbcast(self.onesf[:, 0:1], [TKC, H, 1]))

            with tc.tile_pool(name="ps_kv", bufs=2, space="PSUM") as ps_kv:
                wk = self._load_w(self._wview(wk_d, KC))
                wv = self._load_w(self._wview(wv_d, KC))
                # K/V projections, streaming the source in 512-token slices
                for n in range(NT_FULL):
                    src = tpool.tile([P, KC, 512], F32R, tag="t12")
                    nc.sync.dma_start(out=src[:],
                                      in_=src_r[:, :, n * 512:(n + 1) * 512])
                    if normalize:
                        self._ln_slice(src, src, ps_kv)
                    for co in range(KC):
                        ps = ps_kv.tile([P, 512], F32, tag="proj")
                        for k in range(KC):
                            nc.tensor.matmul(
                                ps, _r(wk[:, k, co * P:(co + 1) * P]),
                                _r(src[:, k, :]),
                                start=(k == 0), stop=(k == KC - 1))
                        self._copyback(
                            kfull[:, co, n * 512:(n + 1) * 512], ps,
                            bk[:, co:co + 1] if bk is not None else None)
                    for ti in range(4):
                        t = 4 * n + ti
                        for hf in range(2):  # 384-wide cout halves (6 heads)
                            ps = ps_kv.tile([P, 384], F32, tag="projv")
                            for k in range(KC):
                                nc.tensor.matmul(
                                    ps, _r(src[:, k, ti * P:(ti + 1) * P]),
                                    _r(wv[:, k, hf * 384:(hf + 1) * 384]),
                                    start=(k == 0), stop=(k == KC - 1))
                            psr = ps.rearrange("p (h d) -> p h d", h=6)
                            dst = vfull[:, t, 6 * hf:6 * hf + 6, 0:DH]
                            if bv_b is not None:
                                bsl = bv_b[:, hf * 384:(hf + 1) * 384].rearrange(
                                    "p (h d) -> p h d", h=6)
                                nc.vector.tensor_tensor(dst, psr, bsl, OP.add)
                            else:
                                nc.vector.tensor_copy(out=dst, in_=psr)

                # Q projection of our own (already-normalized) slice
                wq = self._load_w(self._wview(wq_d, KC))
                h_own = tpool.tile([P, KC, TOWN], F32R, tag="t12")
                nc.vector.tensor_copy(out=h_own[:], in_=xres[:])
                self._ln_slice(h_own, h_own, ps_kv)
                q_sb = tpool.tile([P, KC, TOWN], F32R, tag="t12")
                for co in range(KC):
                    ps = ps_kv.tile([P, 512], F32, tag="proj")
                    for k in range(KC):
                        nc.tensor.matmul(ps, _r(wq[:, k, co * P:(co + 1) * P]),
                                         _r(h_own[:, k, :]),
                                         start=(k == 0), stop=(k == KC - 1))
                    self._copyback(q_sb[:, co, :], ps,
                                   bq[:, co:co + 1] if bq is not None else None)

            # ---- per head: S^T -> exp -> AV (ones-row denominators) ----
            wo = self._load_w(self._wview(wo_d, KC))
            y_sb = tpool.tile([P, KC, TOWN], F32R, tag="t12")
            with tc.tile_pool(name="ps_att", bufs=2, space="PSUM") as ps_att:
                for habs in range(H):
                    co = habs // 2
                    rb0 = DH * (habs % 2)
                    ps_y = ps_att.tile([DH + 1, 512], F32, tag="Yps")
                    kbase = 0
                    for g in SGROUPS:
                        ps_s = ps_att.tile([P, 3, 512], F32, tag="Sps")
                        for i in range(g):
                            kc = kbase + i
                            nc.tensor.matmul(
                                ps_s[:, i, :],
                                _r(kfull[rb0:rb0 + DH, co, kc * P:(kc + 1) * P]),
                                _r(q_sb[rb0:rb0 + DH, co, :]),
                                start=True, stop=True)
                        p_sb = self.ppool.tile([P, 3, 512], F32R, tag="P_sb")
                        # exp(s/sqrt(dh)); no max-subtraction needed here
                        nc.scalar.activation(p_sb[:, 0:g, :], ps_s[:, 0:g, :],
                                             AF.Exp, scale=1.0 / math.sqrt(DH))
                        for i in range(g):
                            kc = kbase + i
                            nc.tensor.matmul(
                                ps_y, _r(vfull[:, kc, habs, :]),
                                _r(p_sb[:, i, :]),
                                start=(kc == 0), stop=(kc == TKC - 1))
                        kbase += g
                    # normalize: y = y_raw / den  (den = ones-row of ps_y)
                    den = self.denpool.tile([1, 512], F32, tag="den")
                    nc.vector.reciprocal(den[:], ps_y[DH:DH + 1, :])
                    # partition-broadcast via a DRAM bounce (stride-0 partition
                    # reads are only legal from DRAM)
                    dden = self.drampool.tile([1, 512], F32, tag="dden")
                    nc.sync.dma_start(out=dden[:], in_=den[:])
                    rb = self.denpool.tile([DH, 512], F32, tag="den")
                    nc.sync.dma_start(out=rb[:], in_=_pbcast(dden[0:1, :], DH))
                    nc.vector.tensor_tensor(y_sb[rb0:rb0 + DH, co, :],
                                            ps_y[0:DH, :], rb[:], OP.mult)

            # ---- output projection, accumulate into residual ----
            with tc.tile_pool(name="ps_out", bufs=2, space="PSUM") as ps_out:
                for co in range(KC):
                    ps = ps_out.tile([P, 512], F32, tag="proj")
                    for k in range(KC):
                        nc.tensor.matmul(ps, _r(wo[:, k, co * P:(co + 1) * P]),
                                         _r(y_sb[:, k, :]),
                                         start=(k == 0), stop=(k == KC - 1))
                    nc.vector.tensor_tensor(xres[:, co, :], xres[:, co, :], ps,
                                            OP.add)
                    if bo is not None:
                        nc.vector.tensor_scalar(xres[:, co, :], xres[:, co, :],
                                                bo[:, co:co + 1], None, OP.add)

    # ---------- main program ----------

    def _build(self):
        nc = self.nc
        dt_in = lambda name, shape: nc.dram_tensor(name, shape, F32,
                                                   kind="ExternalInput")
        dt_inr = lambda name, shape: nc.dram_tensor(name, shape, F32R,
                                                    kind="ExternalInput")
        xT_own = dt_in("xT_own", [C, TOWN])
        xT_full = dt_inr("xT_full", [C, TX])
        ctxT_full = dt_inr("ctxT_full", [C, TC])
        w_sa = {k: dt_inr(f"sa_w{k}", [C, C]) for k in "qkvo"}
        w_xa = {k: dt_inr(f"xa_w{k}", [C, C]) for k in "qkvo"}
        w1 = dt_inr("mlp_w1", [C, H1])
        w2 = dt_inr("mlp_w2", [H1, C])
        out = nc.dram_tensor("outT", [C, TOWN], F32, kind="ExternalOutput")

        with tile.TileContext(nc) as tc, contextlib.ExitStack() as ctx:
            pool = lambda name, bufs, **kw: ctx.enter_context(
                tc.tile_pool(name=name, bufs=bufs, **kw))
            self.gpool = pool("gmisc", 1)
            self.wpool = pool("weights", 2)
            self.lntmp = pool("lntmp", 2)
            self.ppool = pool("psb", 2)
            self.denpool = pool("den", 2)
            self.biaspool = pool("bias", 1)
            self.drampool = pool("dram", 2, space="DRAM")

            # memset can't write float32r; build ones via a rounding DVE copy
            self.onesf = self.gpool.tile([P, 1], F32, tag="onesf")
            nc.vector.memset(self.onesf[:], 1.0)
            self.ones = self.gpool.tile([P, P], F32R, tag="ones")
            nc.vector.tensor_copy(out=self.ones[:],
                                  in_=_fbcast(self.onesf[:, 0:1], [P]))

            # Residual stream (feature-major) for our 512 tokens
            xres = self.gpool.tile([P, KC, TOWN], F32, tag="xres")
            nc.sync.dma_start(
                out=xres[:],
                in_=xT_own.ap().rearrange("(ko p) t -> p ko t", p=P))

            biases = {}
            for pre in ("sa", "xa"):
                for k in "qko":
                    biases[f"{pre}_b{k}"] = self._bias_cols(f"{pre}_b{k}", KC)
            b1_cols = self._bias_cols("mlp_b1", MC1)
            b2_cols = self._bias_cols("mlp_b2", KC)

            # ================= Self-attention =================
            self._attn_stage(tc, xT_full, True, w_sa["q"], w_sa["k"],
                             w_sa["v"], w_sa["o"], biases["sa_bq"],
                             biases["sa_bk"], "sa_bv", biases["sa_bo"], xres)

            # ================= Cross-attention =================
            self._attn_stage(tc, ctxT_full, False, w_xa["q"], w_xa["k"],
                             w_xa["v"], w_xa["o"], biases["xa_bq"],
                             biases["xa_bk"], "xa_bv", biases["xa_bo"], xres)

            # ===================== MLP =====================
            with contextlib.ExitStack() as st:
                mpool = st.enter_context(tc.tile_pool(name="mlp", bufs=1))
                ps_m = st.enter_context(
                    tc.tile_pool(name="ps_mlp", bufs=2, space="PSUM"))
                h3 = mpool.tile([P, KC, TOWN], F32R, tag="h3")
                nc.vector.tensor_copy(out=h3[:], in_=xres[:])
                self._ln_slice(h3, h3, ps_m)

                g_sb = mpool.tile([P, MC1, TOWN], F32R, tag="g_sb")
                w1_r = self._wview(w1, KC)
                for mo in range(6):  # 24 hidden chunks in groups of 4
                    w1s = self._load_w(w1_r[:, :, mo * 512:(mo + 1) * 512])
                    for mi in range(4):
                        m = 4 * mo + mi
                        ps = ps_m.tile([P, 512], F32, tag="proj")
                        for k in range(KC):
                            nc.tensor.matmul(
                                ps, _r(w1s[:, k, mi * P:(mi + 1) * P]),
                                _r(h3[:, k, :]),
                                start=(k == 0), stop=(k == KC - 1))
                        # exact (erf) GELU with fused pre-bias
                        nc.scalar.activation(
                            g_sb[:, m, :], ps, AF.Gelu,
                            bias=b1_cols[:, m:m + 1] if b1_cols is not None
                            else 0.0)
                w2_r = self._wview(w2, MC1)
                for co in range(KC):
                    w2s = self._load_w(w2_r[:, :, co * P:(co + 1) * P])
                    ps = ps_m.tile([P, 512], F32, tag="proj")
                    for k in range(MC1):
                        nc.tensor.matmul(ps, _r(w2s[:, k, :]),
                                         _r(g_sb[:, k, :]),
                                         start=(k == 0), stop=(k == MC1 - 1))
                    nc.vector.tensor_tensor(xres[:, co, :], xres[:, co, :], ps,
                                            OP.add)
                    if b2_cols is not None:
                        nc.vector.tensor_scalar(xres[:, co, :], xres[:, co, :],
                                                b2_cols[:, co:co + 1], None,
                                                OP.add)

            # ==================== Output ====================
            nc.sync.dma_start(
                out=out.ap().rearrange("(ko p) t -> p ko t", p=P),
                in_=xres[:])
        nc.compile()


def _fold_ln(w, b, g, lb):
    """Fold layernorm gain/bias into the following projection: (ln0(x)*g+lb)@W
    + b == ln0(x) @ (g[:,None]*W) + (lb @ W + b)."""
    w = np.asarray(w, np.float32)
    b = np.asarray(b, np.float32)
    g = np.asarray(g, np.float32)
    lb = np.asarray(lb, np.float32)
    return (g[:, None] * w).astype(np.float32), (lb @ w + b).astype(np.float32)


_PROG_CACHE = {}


def _get_prog(bias_nz):
    key = tuple(sorted(bias_nz.items()))
    if key not in _PROG_CACHE:
        _PROG_CACHE[key] = _Prog(bias_nz)
    return _PROG_CACHE[key]


def _prepare(inputs):
    """Host-side prep: fold LN into weights, transpose activations, build the
    8 per-core input maps. Returns (bias_nz, in_maps, x, context)."""
    inp = {k: np.asarray(v) for k, v in inputs.items()}
    n_head = int(inp["n_head"])
    assert n_head == H, f"kernel hardcoded for {H} heads, got {n_head}"
    x = inp["x"].astype(np.float32)            # [B, TX, C]
    context = inp["context"].astype(np.float32)

    w, bvec = {}, {}
    for k in "qkv":
        w[f"sa_w{k}"], bvec[f"sa_b{k}"] = _fold_ln(
            inp[f"sa_w{k}"], inp[f"sa_b{k}"], inp["ln1_g"], inp["ln1_b"])
    w["sa_wo"], bvec["sa_bo"] = (np.asarray(inp["sa_wo"], np.float32),
                                 np.asarray(inp["sa_bo"], np.float32))
    w["xa_wq"], bvec["xa_bq"] = _fold_ln(
        inp["xa_wq"], inp["xa_bq"], inp["ln2_g"], inp["ln2_b"])
    for k in "kv":  # context is NOT normalized in the reference
        w[f"xa_w{k}"], bvec[f"xa_b{k}"] = (
            np.asarray(inp[f"xa_w{k}"], np.float32),
            np.asarray(inp[f"xa_b{k}"], np.float32))
    w["xa_wo"], bvec["xa_bo"] = (np.asarray(inp["xa_wo"], np.float32),
                                 np.asarray(inp["xa_bo"], np.float32))
    w["mlp_w1"], bvec["mlp_b1"] = _fold_ln(
        inp["mlp_w1"], inp["mlp_b1"], inp["ln3_g"], inp["ln3_b"])
    w["mlp_w2"] = np.asarray(inp["mlp_w2"], np.float32)
    bvec["mlp_b2"] = np.asarray(inp["mlp_b2"], np.float32)

    bias_nz = {name: bool(np.any(v)) for name, v in bvec.items()}

    xT = np.ascontiguousarray(x.transpose(0, 2, 1))        # [B, C, TX]
    ctxT = np.ascontiguousarray(context.transpose(0, 2, 1))

    common = {name: np.ascontiguousarray(arr) for name, arr in w.items()}
    for name, vec in bvec.items():
        if bias_nz[name]:
            common[name] = np.ascontiguousarray(vec.astype(np.float32))

    in_maps = []
    for core in range(8):
        b, s = divmod(core, 4)
        m = dict(common)
        m["xT_full"] = xT[b]
        m["ctxT_full"] = ctxT[b]
        m["xT_own"] = np.ascontiguousarray(xT[b][:, s * TOWN:(s + 1) * TOWN])
        in_maps.append(m)
    return bias_nz, in_maps, x, context


def _gather(results, x):
    x_out = np.empty_like(x)
    for core in range(8):
        b, s = divmod(core, 4)
        x_out[b, s * TOWN:(s + 1) * TOWN, :] = results[core]["outT"].T
    return x_out


def kernel(**inputs):
    bias_nz, in_maps, x, context = _prepare(inputs)
    prog = _get_prog(bias_nz)
    res = run_bass_kernel_spmd(prog.nc, in_maps, core_ids=list(range(8)))
    return (_gather(res.results, x), context)
